# revision 1
# baseline (speedup 1.0000x reference)
"""LoRA linear kernel for Trainium2 (Bass/Tile), 8-core SPMD.

Computes out = x @ (A @ B) * (alpha/r) for
  x: [4, 4096, 4096] f32, A: [4096, 16] f32, B: [16, 4096] f32
with alpha/r == 1.0.

Algorithm: reassociate as out = (x @ A) @ B  -- 128x fewer FLOPs than
materializing the 4096x4096 delta-weight.  Data-parallel over rows of x:
each of the 8 cores gets 2048 rows.

Per-core pipeline (m processed in groups of MG m-tiles of 128 rows):
  1. DMA x rows in natural layout [m=128p, k].
  2. PE transpose 128x128 blocks (x_blk.T via identity matmul) -> PSUM.
  3. DVE copy PSUM -> SBUF (xT chunk, [k=128p, m]).
  4. matmul1: tT[r, m] += A_chunk[k,r].T @ xT_chunk[k, m]  (accum over k).
  5. matmul2: out[m, n] = tT[:, m].T @ B[r, n] in 512-col chunks.
  6. DVE/ACT copy PSUM -> SBUF, DMA out.
"""

import os
import sys

import numpy as np

for _p in ("/opt/trn_rl_repo",):
    if os.path.isdir(_p) and _p not in sys.path:
        sys.path.insert(0, _p)

import concourse.bacc as bacc
import concourse.bass as bass
import concourse.mybir as mybir
from concourse import tile
from concourse.alu_op_type import AluOpType
from concourse.bass_utils import run_bass_kernel_spmd

import ml_dtypes

R = 16
B_DIM = 4
SEQ = 4096
K = 4096  # in_features
N = 4096  # out_features
M_FULL = B_DIM * SEQ  # 16384
NCORES = 8
M_SHARD = M_FULL // NCORES  # 2048
SCALING = 16.0 / 16.0  # alpha / r == 1.0

MT = 128  # rows per m-tile
MG = 2  # m-tiles per group (transpose/mm1 free dim = MG*MT = 256)
KC = 128  # contraction chunk
N_CHUNK = 512  # matmul2 output chunk (one PSUM bank of fp32)

# Use the fp32 fast-path dtype for matmul operands (1 cyc/row at N>=256
# instead of 4 cyc/row for plain fp32). Flip off if numerics are bad.
F32R = os.environ.get("KERNEL_F32R", "0") == "1"

_F32 = mybir.dt.float32
_F32R = mybir.dt.float32r
_BF16 = mybir.dt.bfloat16


def _mm_cast(ap):
    return ap.bitcast(_F32R) if F32R else ap


def _build_kernel(tc, nc, x, a_pre, b_in, ident_d, out):
    n_groups = M_SHARD // (MT * MG)  # 8
    n_kc = K // KC  # 32
    n_nc = N // N_CHUNK  # 8

    with (
        tc.tile_pool(name="const", bufs=1) as cpool,
        tc.tile_pool(name="xin", bufs=4) as xpool,
        tc.tile_pool(name="xtps", bufs=3, space="PSUM") as xtpsum,
        tc.tile_pool(name="xts", bufs=3) as xtpool,
        tc.tile_pool(name="tps", bufs=2, space="PSUM") as tpsum,
        tc.tile_pool(name="tsb", bufs=2) as tspool,
        tc.tile_pool(name="ops", bufs=3, space="PSUM") as opsum,
        tc.tile_pool(name="osb", bufs=3) as opool,
    ):
        ident = cpool.tile([128, 128], _F32, name="ident")
        nc.sync.dma_start(out=ident, in_=ident_d)
        # A pre-arranged on host to [128, n_kc * R]: col block c holds
        # A[c*128:(c+1)*128, :] with k on partitions.
        a_sb = cpool.tile([128, n_kc * R], _F32, name="a_sb")
        nc.sync.dma_start(out=a_sb, in_=a_pre)
        # Rounded copy of A for the single-pass fp32 (FP32r) matmul1 path.
        a_f32r = cpool.tile([128, n_kc * R], _F32R, name="a_f32r")
        nc.vector.tensor_copy(a_f32r[:], a_sb[:])
        # B stacked on host in 32-aligned bands (bf16): rows 0-15 Bh,
        # 32-47 Bh, 64-79 Bl, other bands zero.  With t split as
        # th(@0) / tl(@32) / th(@64), one K=96 bf16 matmul computes
        # t @ B ~= (th + tl) @ Bh + th @ Bl  (drops only tl @ Bl ~ 2^-18).
        b_sb = cpool.tile([96, N], _BF16, name="b_sb")
        nc.sync.dma_start(out=b_sb, in_=b_in)

        for g in range(n_groups):
            xg = []
            for mi in range(MG):
                xt = xpool.tile([MT, K], _F32)
                row0 = (g * MG + mi) * MT
                nc.sync.dma_start(out=xt, in_=x[row0 : row0 + MT, :])
                xg.append(xt)

            tps = tpsum.tile([R, MT * MG], _F32)
            for c in range(n_kc):
                xtp = xtpsum.tile([128, MT * MG], _F32)
                for mi in range(MG):
                    nc.tensor.transpose(
                        _mm_cast(xtp[:, mi * MT : (mi + 1) * MT]),
                        _mm_cast(xg[mi][:, c * KC : (c + 1) * KC]),
                        _mm_cast(ident[:]),
                    )
                # PSUM->SBUF copy rounds to FP32r on the way, making xts a
                # legal operand for the single-pass FP32r matmul1.
                xts = xtpool.tile([128, MT * MG], _F32R)
                nc.vector.tensor_copy(xts[:], xtp[:])
                nc.tensor.matmul(
                    tps[:],
                    a_f32r[:, c * R : (c + 1) * R],
                    xts[:],
                    start=(c == 0),
                    stop=(c == n_kc - 1),
                )

            # t split into bf16 hi/lo at 32-aligned partition bands
            # (engine writes must start at partition 0/32/64/96).
            ts = tspool.tile([96, MT * MG], _BF16)
            nc.gpsimd.memset(ts[:], 0.0)
            nc.vector.tensor_copy(ts[0:R, :], tps[:])
            nc.vector.tensor_tensor(
                ts[32 : 32 + R, :], tps[:], ts[0:R, :], op=AluOpType.subtract
            )
            nc.vector.tensor_copy(ts[64 : 64 + R, :], ts[0:R, :])

            for mi in range(MG):
                osb = opool.tile([MT, N], _F32)
                for j in range(n_nc):
                    ops = opsum.tile([MT, N_CHUNK], _F32)
                    nc.tensor.matmul(
                        ops[:],
                        ts[:, mi * MT : (mi + 1) * MT],
                        b_sb[:, j * N_CHUNK : (j + 1) * N_CHUNK],
                        start=True,
                        stop=True,
                    )
                    dst = osb[:, j * N_CHUNK : (j + 1) * N_CHUNK]
                    nc.scalar.copy(dst, ops[:])
                row0 = (g * MG + mi) * MT
                nc.sync.dma_start(out=out[row0 : row0 + MT, :], in_=osb[:])


_NC_CACHE = None


def _get_nc():
    global _NC_CACHE
    if _NC_CACHE is not None:
        return _NC_CACHE
    nc = bacc.Bacc("TRN2", target_bir_lowering=False, debug=False)
    x = nc.dram_tensor("x", [M_SHARD, K], _F32, kind="ExternalInput").ap()
    a_pre = nc.dram_tensor("a_pre", [128, (K // KC) * R], _F32, kind="ExternalInput").ap()
    b_in = nc.dram_tensor("b_in", [96, N], _BF16, kind="ExternalInput").ap()
    ident_d = nc.dram_tensor("ident", [128, 128], _F32, kind="ExternalInput").ap()
    out = nc.dram_tensor("out", [M_SHARD, N], _F32, kind="ExternalOutput").ap()
    with tile.TileContext(nc) as tc:
        _build_kernel(tc, nc, x, a_pre, b_in, ident_d, out)
    nc.compile()
    _NC_CACHE = nc
    return nc


LAST_RESULTS = None


def kernel(x: np.ndarray, A: np.ndarray, B: np.ndarray) -> np.ndarray:
    global LAST_RESULTS
    assert x.shape == (B_DIM, SEQ, K), x.shape
    assert A.shape == (K, R), A.shape
    assert B.shape == (R, N), B.shape

    x_np = np.ascontiguousarray(np.asarray(x, dtype=np.float32))
    a_np = np.asarray(A, dtype=np.float32)
    b_f32 = np.asarray(B, dtype=np.float32) * SCALING
    b_hi = b_f32.astype(ml_dtypes.bfloat16)
    b_lo = (b_f32 - b_hi.astype(np.float32)).astype(ml_dtypes.bfloat16)
    b_np = np.zeros((96, N), dtype=ml_dtypes.bfloat16)
    b_np[0:R] = b_hi
    b_np[32 : 32 + R] = b_hi
    b_np[64 : 64 + R] = b_lo

    # Host pre-arrangement of A: [K, R] -> [128, (K/128) * R]
    a_pre = np.ascontiguousarray(
        a_np.reshape(K // KC, KC, R).transpose(1, 0, 2).reshape(128, (K // KC) * R)
    )
    ident = np.eye(128, dtype=np.float32)

    x_flat = x_np.reshape(M_FULL, K)
    in_maps = []
    for i in range(NCORES):
        in_maps.append(
            {
                "x": x_flat[i * M_SHARD : (i + 1) * M_SHARD],
                "a_pre": a_pre,
                "b_in": b_np,
                "ident": ident,
            }
        )

    nc = _get_nc()
    trace = os.environ.get("KERNEL_TRACE", "0") == "1"
    tmpdir = os.environ.get("KERNEL_TMPDIR") or None
    res = run_bass_kernel_spmd(
        nc, in_maps, core_ids=list(range(NCORES)), trace=trace, tmpdir=tmpdir
    )
    LAST_RESULTS = res
    out = np.concatenate([res.results[i]["out"] for i in range(NCORES)], axis=0)
    return out.reshape(B_DIM, SEQ, N)

